# revision 1
# baseline (speedup 1.0000x reference)
"""Trainium2 Bass kernel for nn_AttnBlock (linear-attention block).

Full-input contract: kernel(**inputs) takes the complete arrays and returns the
complete output. Internally shards batch B=16 across 8 NeuronCores (2 each).

Math (per batch b, x_b [C=256, N=4096]):
  n1 = LN_C(x);  qkv = Wqkv @ n1;  q,k,v heads of 32
  q = softmax_d(q)/sqrt(32); k = softmax_N(k); v = v/N
  ctx_h = k_h @ v_h^T; out_h = ctx_h^T @ q_h
  y = Wout @ out + bout; out = LN_C(y) + x

Key folds used on-device:
  - LN mean-subtraction folded into host-centered weights:
      Wqkv@((x-mu)*rs) = (Wqkv - rowmean(Wqkv)) @ (x*rs)
      and LN2's centering into column-centered Wout/bout, so LN2 var = E[y_c^2].
  - partition-axis sums via PE matmul with an all-(1/256) lhsT, which also
    broadcasts the stat to all 128 partitions in the same pass.
  - k-softmax denominator via ACT accum_out on the Exp op (free row-sum);
    its reciprocal is applied to the tiny [128,128] context matrix instead of
    the [128,4096] k tensor.
  - 1/N and 1/sqrt(32) folded into the per-head block mask applied to context.
"""

import math
import numpy as np

HEADS = 4
DH = 32
C = 256
N = 4096
B = 16
NCORES = 8
BPC = B // NCORES  # batches per core
EPS = 1e-5
INNER = HEADS * DH  # 128
F32 = None  # set after mybir import


def _build_bass():
    import concourse.bass as bass
    import concourse.bacc as bacc
    import concourse.tile as tile
    import concourse.mybir as mybir
    from contextlib import ExitStack

    f32 = mybir.dt.float32
    AF = mybir.ActivationFunctionType
    ALU = mybir.AluOpType
    AX = mybir.AxisListType

    nc = bacc.Bacc("TRN2", target_bir_lowering=False, debug=False,
                   num_devices=NCORES)

    # DRAM I/O
    xin = nc.dram_tensor("xin", [BPC, C, N], f32, kind="ExternalInput")
    wct = nc.dram_tensor("wct", [C, 3 * INNER], f32, kind="ExternalInput")
    woct = nc.dram_tensor("woct", [INNER, C], f32, kind="ExternalInput")
    boc = nc.dram_tensor("boc", [C, 1], f32, kind="ExternalInput")
    onesc = nc.dram_tensor("onesc", [128, 128], f32, kind="ExternalInput")
    hind = nc.dram_tensor("hind", [128, 128], f32, kind="ExternalInput")
    bmask = nc.dram_tensor("bmask", [128, 128], f32, kind="ExternalInput")
    ident = nc.dram_tensor("ident", [128, 128], f32, kind="ExternalInput")
    out = nc.dram_tensor("out", [BPC, C, N], f32, kind="ExternalOutput")

    with tile.TileContext(nc) as tc, ExitStack() as ctx:
        consts = ctx.enter_context(tc.tile_pool(name="consts", bufs=1))
        xpool = ctx.enter_context(tc.tile_pool(name="xpool", bufs=2))
        sqpool = ctx.enter_context(tc.tile_pool(name="sqpool", bufs=4))
        eqpool = ctx.enter_context(tc.tile_pool(name="eqpool", bufs=1))
        ycpool = ctx.enter_context(tc.tile_pool(name="ycpool", bufs=2))
        rspool = ctx.enter_context(tc.tile_pool(name="rspool", bufs=1))
        statp = ctx.enter_context(tc.tile_pool(name="statp", bufs=2))
        smallp = ctx.enter_context(tc.tile_pool(name="smallp", bufs=2))
        outp = ctx.enter_context(tc.tile_pool(name="outp", bufs=2))
        tinyp = ctx.enter_context(tc.tile_pool(name="tinyp", bufs=4))
        psA = ctx.enter_context(tc.tile_pool(name="psA", bufs=6, space="PSUM"))
        psC = ctx.enter_context(tc.tile_pool(name="psC", bufs=1, space="PSUM"))

        # constants into SBUF once
        wct_t = []
        for kt in range(2):
            t = consts.tile([128, 3 * INNER], f32, tag=f"wct{kt}")
            nc.sync.dma_start(t[:], wct[kt * 128:(kt + 1) * 128, :])
            wct_t.append(t)
        woct_t = consts.tile([128, C], f32, tag="woct")
        nc.sync.dma_start(woct_t[:], woct[:, :])
        boc_t = []
        for j in range(2):
            t = consts.tile([128, 1], f32, tag=f"boc{j}")
            nc.sync.dma_start(t[:], boc[j * 128:(j + 1) * 128, :])
            boc_t.append(t)
        ones_t = consts.tile([128, 128], f32, tag="ones")
        nc.sync.dma_start(ones_t[:], onesc[:, :])
        hind_t = consts.tile([128, 128], f32, tag="hind")
        nc.sync.dma_start(hind_t[:], hind[:, :])
        bmask_t = consts.tile([128, 128], f32, tag="bmask")
        nc.sync.dma_start(bmask_t[:], bmask[:, :])
        id_t = consts.tile([128, 128], f32, tag="ident")
        nc.sync.dma_start(id_t[:], ident[:, :])
        eps_t = consts.tile([128, 1], f32, tag="eps")
        nc.vector.memset(eps_t[:], EPS)

        # PE "warm-up" touch of every constant: each matmul waits on exactly
        # one DMA lane, so no later PE instruction needs >1 sync wait.
        warm_ps = psA.tile([128, 128], f32, tag="pa")
        for t in (wct_t[0], wct_t[1], woct_t, ones_t, hind_t, bmask_t, id_t):
            nc.tensor.matmul(warm_ps[:, 0:1], t[:, 0:128], t[:, 0:1],
                             start=True, stop=True)
        for t in boc_t:
            nc.tensor.matmul(warm_ps[0:1, 0:1], t[:, 0:1], t[:, 0:1],
                             start=True, stop=True)

        NCH = 8          # 512-wide chunks
        CW = N // NCH    # 512

        for b in range(BPC):
            # ---- load x (2 c-tiles) ----
            xa = xpool.tile([128, N], f32, tag="x")
            xb = xpool.tile([128, N], f32, tag="x")
            nc.sync.dma_start(xa[:], xin[b, 0:128, :])
            nc.sync.dma_start(xb[:], xin[b, 128:256, :])

            # ---- LN1 stats -> rsig [128, N] broadcast ----
            rsig = rspool.tile([128, N], f32, tag="rsig")
            for ch in range(NCH):
                sl = bass.ts(ch, CW)
                xsq_a = sqpool.tile([128, CW], f32, tag="sq")
                xsq_b = sqpool.tile([128, CW], f32, tag="sq")
                nc.scalar.activation(xsq_a[:], xa[:, sl], AF.Square)
                nc.scalar.activation(xsq_b[:], xb[:, sl], AF.Square)
                mu_ps = psA.tile([128, CW], f32, tag="pa")
                nc.tensor.matmul(mu_ps[:], ones_t[:], xa[:, sl], start=True, stop=False)
                nc.tensor.matmul(mu_ps[:], ones_t[:], xb[:, sl], start=False, stop=True)
                msq_ps = psA.tile([128, CW], f32, tag="pa")
                nc.tensor.matmul(msq_ps[:], ones_t[:], xsq_a[:], start=True, stop=False)
                nc.tensor.matmul(msq_ps[:], ones_t[:], xsq_b[:], start=False, stop=True)
                musq = statp.tile([128, CW], f32, tag="st1")
                nc.scalar.activation(musq[:], mu_ps[:], AF.Square)
                var = statp.tile([128, CW], f32, tag="st2")
                nc.vector.tensor_tensor(var[:], msq_ps[:], musq[:], op=ALU.subtract)
                sd = statp.tile([128, CW], f32, tag="st3")
                nc.scalar.activation(sd[:], var[:], AF.Sqrt, bias=eps_t[:])
                nc.vector.reciprocal(rsig[:, sl], sd[:])

            # ---- qkv + exp + transposes + context accumulation ----
            expq = eqpool.tile([128, N], f32, tag="eq")
            ksum_parts = tinyp.tile([128, NCH], f32, tag="ksp")
            ctx_ps = psC.tile([128, 128], f32, tag="ctx")
            for ch in range(NCH):
                sl = bass.ts(ch, CW)
                xs_a = smallp.tile([128, CW], f32, tag="xs")
                xs_b = smallp.tile([128, CW], f32, tag="xs")
                nc.vector.tensor_mul(xs_a[:], xa[:, sl], rsig[:, sl])
                nc.vector.tensor_mul(xs_b[:], xb[:, sl], rsig[:, sl])

                q_ps = psA.tile([128, CW], f32, tag="pa")
                nc.tensor.matmul(q_ps[:], wct_t[0][:, 0:128], xs_a[:], start=True, stop=False)
                nc.tensor.matmul(q_ps[:], wct_t[1][:, 0:128], xs_b[:], start=False, stop=True)
                k_ps = psA.tile([128, CW], f32, tag="pa")
                nc.tensor.matmul(k_ps[:], wct_t[0][:, 128:256], xs_a[:], start=True, stop=False)
                nc.tensor.matmul(k_ps[:], wct_t[1][:, 128:256], xs_b[:], start=False, stop=True)
                v_ps = psA.tile([128, CW], f32, tag="pa")
                nc.tensor.matmul(v_ps[:], wct_t[0][:, 256:384], xs_a[:], start=True, stop=False)
                nc.tensor.matmul(v_ps[:], wct_t[1][:, 256:384], xs_b[:], start=False, stop=True)

                nc.scalar.activation(expq[:, sl], q_ps[:], AF.Exp)
                expk = smallp.tile([128, CW], f32, tag="ek")
                nc.scalar.activation(expk[:], k_ps[:], AF.Exp,
                                     accum_out=ksum_parts[:, ch:ch + 1])
                v_sb = smallp.tile([128, CW], f32, tag="vv")
                nc.scalar.copy(v_sb[:], v_ps[:])

                kT_ps = psA.tile([128, CW], f32, tag="pa")
                vT_ps = psA.tile([128, CW], f32, tag="pa")
                for j in range(4):
                    jl = bass.ts(j, 128)
                    nc.tensor.transpose(kT_ps[:, jl], expk[:, jl], id_t[:])
                    nc.tensor.transpose(vT_ps[:, jl], v_sb[:, jl], id_t[:])
                kT = smallp.tile([128, CW], f32, tag="kt")
                nc.vector.tensor_copy(kT[:], kT_ps[:])
                vT = smallp.tile([128, CW], f32, tag="vt")
                nc.vector.tensor_copy(vT[:], vT_ps[:])
                for j in range(4):
                    jl = bass.ts(j, 128)
                    nc.tensor.matmul(ctx_ps[:], kT[:, jl], vT[:, jl],
                                     start=(ch == 0 and j == 0),
                                     stop=(ch == NCH - 1 and j == 3))

            # ---- finish context: apply 1/ksum rows and scaled head mask ----
            ksum = tinyp.tile([128, 1], f32, tag="ks1")
            nc.vector.tensor_reduce(ksum[:], ksum_parts[:], axis=AX.X, op=ALU.add)
            rk = tinyp.tile([128, 1], f32, tag="rk")
            nc.vector.reciprocal(rk[:], ksum[:])
            ctx_a = tinyp.tile([128, 128], f32, tag="cxa")
            nc.vector.tensor_scalar_mul(ctx_a[:], ctx_ps[:], rk[:])
            ctx_m = tinyp.tile([128, 128], f32, tag="cxm")
            nc.vector.tensor_mul(ctx_m[:], ctx_a[:], bmask_t[:])

            # ---- q normalization + out einsum + Wout ----
            yc_a = ycpool.tile([128, N], f32, tag="yc")
            yc_b = ycpool.tile([128, N], f32, tag="yc")
            for ch in range(NCH):
                sl = bass.ts(ch, CW)
                S_ps = psA.tile([128, CW], f32, tag="pa")
                nc.tensor.matmul(S_ps[:], hind_t[:], expq[:, sl], start=True, stop=True)
                rS = smallp.tile([128, CW], f32, tag="rs")
                nc.vector.reciprocal(rS[:], S_ps[:])
                o_ps = psA.tile([128, CW], f32, tag="pa")
                nc.tensor.matmul(o_ps[:], ctx_m[:], expq[:, sl], start=True, stop=True)
                attn = smallp.tile([128, CW], f32, tag="at")
                nc.vector.tensor_mul(attn[:], o_ps[:], rS[:])
                y_ps0 = psA.tile([128, CW], f32, tag="pa")
                nc.tensor.matmul(y_ps0[:], woct_t[:, 0:128], attn[:], start=True, stop=True)
                y_ps1 = psA.tile([128, CW], f32, tag="pa")
                nc.tensor.matmul(y_ps1[:], woct_t[:, 128:256], attn[:], start=True, stop=True)
                nc.scalar.activation(yc_a[:, sl], y_ps0[:], AF.Identity, bias=boc_t[0][:])
                nc.scalar.activation(yc_b[:, sl], y_ps1[:], AF.Identity, bias=boc_t[1][:])

            # ---- LN2 (centered by construction) + residual ----
            for ch in range(NCH):
                sl = bass.ts(ch, CW)
                ysq_a = sqpool.tile([128, CW], f32, tag="sq")
                ysq_b = sqpool.tile([128, CW], f32, tag="sq")
                nc.scalar.activation(ysq_a[:], yc_a[:, sl], AF.Square)
                nc.scalar.activation(ysq_b[:], yc_b[:, sl], AF.Square)
                m2_ps = psA.tile([128, CW], f32, tag="pa")
                nc.tensor.matmul(m2_ps[:], ones_t[:], ysq_a[:], start=True, stop=False)
                nc.tensor.matmul(m2_ps[:], ones_t[:], ysq_b[:], start=False, stop=True)
                sd2 = statp.tile([128, CW], f32, tag="st1")
                nc.scalar.activation(sd2[:], m2_ps[:], AF.Sqrt, bias=eps_t[:])
                rsig2 = statp.tile([128, CW], f32, tag="st2")
                nc.vector.reciprocal(rsig2[:], sd2[:])
                t_a = statp.tile([128, CW], f32, tag="st3")
                t_b = statp.tile([128, CW], f32, tag="st4")
                nc.vector.tensor_mul(t_a[:], yc_a[:, sl], rsig2[:])
                nc.vector.tensor_mul(t_b[:], yc_b[:, sl], rsig2[:])
                o_a = outp.tile([128, CW], f32, tag="oa")
                o_b = outp.tile([128, CW], f32, tag="ob")
                nc.vector.tensor_add(o_a[:], t_a[:], xa[:, sl])
                nc.vector.tensor_add(o_b[:], t_b[:], xb[:, sl])
                nc.sync.dma_start(out[b, 0:128, sl], o_a[:])
                nc.sync.dma_start(out[b, 128:256, sl], o_b[:])

    nc.compile()
    return nc


_CACHED = {}


def _get_nc():
    if "nc" not in _CACHED:
        _CACHED["nc"] = _build_bass()
    return _CACHED["nc"]


def kernel(x, Wqkv, Wout, bout):
    from concourse.bass_utils import run_bass_kernel_spmd

    x = np.ascontiguousarray(x, dtype=np.float32)
    Wqkv = np.asarray(Wqkv, dtype=np.float32)
    Wout = np.asarray(Wout, dtype=np.float32)
    bout = np.asarray(bout, dtype=np.float32)

    # host-side weight folding
    Wc = Wqkv - Wqkv.mean(axis=1, keepdims=True)          # centers LN1 input
    wct = np.ascontiguousarray(Wc.T)                      # [256, 384]
    Woc = Wout - Wout.mean(axis=0, keepdims=True)         # centers LN2 input
    woct = np.ascontiguousarray(Woc.T)                    # [128, 256]
    boc = (bout - bout.mean()).reshape(C, 1).astype(np.float32)

    onesc = np.full((128, 128), 1.0 / C, dtype=np.float32)
    r = np.arange(128)
    hind = (r[:, None] // DH == r[None, :] // DH).astype(np.float32)
    bmask = hind * np.float32(1.0 / (N * math.sqrt(DH)))
    ident = np.eye(128, dtype=np.float32)

    xr = x.reshape(B, C, N)
    nc = _get_nc()
    in_maps = []
    for core in range(NCORES):
        in_maps.append({
            "xin": np.ascontiguousarray(xr[core * BPC:(core + 1) * BPC]),
            "wct": wct, "woct": woct, "boc": boc,
            "onesc": onesc, "hind": hind, "bmask": bmask, "ident": ident,
        })
    res = run_bass_kernel_spmd(nc, in_maps, core_ids=list(range(NCORES)))
    outs = [res.results[c]["out"] for c in range(NCORES)]
    full = np.concatenate(outs, axis=0).reshape(B, C, 64, 64)
    return full


if __name__ == "__main__":
    rng = np.random.default_rng(0)
    x = rng.standard_normal((B, C, 64, 64), dtype=np.float32)
    Wqkv = rng.standard_normal((3 * INNER, C), dtype=np.float32)
    Wout = rng.standard_normal((C, INNER), dtype=np.float32)
    bout = rng.standard_normal((C,), dtype=np.float32)
    y = kernel(x=x, Wqkv=Wqkv, Wout=Wout, bout=bout)
    print(y.shape, y.dtype)



# revision 11
# speedup vs baseline: 1.4201x; 1.4201x over previous
"""Trainium2 Bass kernel for nn_AttnBlock (linear-attention block).

Full-input contract: kernel(**inputs) takes the complete arrays and returns the
complete output. Internally shards batch B=16 across 8 NeuronCores (2 each).

Math (per batch b, x_b [C=256, N=4096]):
  n1 = LN_C(x);  qkv = Wqkv @ n1;  q,k,v heads of 32
  q = softmax_d(q)/sqrt(32); k = softmax_N(k); v = v/N
  ctx_h = k_h @ v_h^T; out_h = ctx_h^T @ q_h
  y = Wout @ out + bout; out = LN_C(y) + x

Key folds used on-device:
  - LN mean-subtraction folded into host-centered weights:
      Wqkv@((x-mu)*rs) = (Wqkv - rowmean(Wqkv)) @ (x*rs)
      and LN2's centering into column-centered Wout/bout, so LN2 var = E[y_c^2].
  - partition-axis sums via PE matmul with an all-(1/256) lhsT, which also
    broadcasts the stat to all 128 partitions in the same pass.
  - k-softmax denominator via ACT accum_out on the Exp op (free row-sum);
    its reciprocal is applied to the tiny [128,128] context matrix instead of
    the [128,4096] k tensor.
  - 1/N and 1/sqrt(32) folded into the per-head block mask applied to context.
"""

import math
import numpy as np

HEADS = 4
DH = 32
C = 256
N = 4096
B = 16
NCORES = 8
BPC = B // NCORES  # batches per core
EPS = 1e-5
INNER = HEADS * DH  # 128
F32 = None  # set after mybir import


def _build_bass():
    import concourse.bass as bass
    import concourse.bacc as bacc
    import concourse.tile as tile
    import concourse.mybir as mybir
    from contextlib import ExitStack

    f32 = mybir.dt.float32
    f32r = mybir.dt.float32r
    AF = mybir.ActivationFunctionType
    ALU = mybir.AluOpType
    AX = mybir.AxisListType

    nc = bacc.Bacc("TRN2", target_bir_lowering=False, debug=False,
                   num_devices=NCORES)

    # DRAM I/O
    xin = nc.dram_tensor("xin", [BPC, C, N], f32r, kind="ExternalInput")
    wct = nc.dram_tensor("wct", [C, 3 * INNER], f32r, kind="ExternalInput")
    woct = nc.dram_tensor("woct", [INNER, C], f32r, kind="ExternalInput")
    boc = nc.dram_tensor("boc", [C, 1], f32, kind="ExternalInput")
    onesc = nc.dram_tensor("onesc", [128, 128], f32r, kind="ExternalInput")
    hind = nc.dram_tensor("hind", [128, 128], f32r, kind="ExternalInput")
    bmask = nc.dram_tensor("bmask", [128, 128], f32, kind="ExternalInput")
    ident = nc.dram_tensor("ident", [128, 128], f32r, kind="ExternalInput")
    out = nc.dram_tensor("out", [BPC, C, N], f32, kind="ExternalOutput")

    with tile.TileContext(nc) as tc, ExitStack() as ctx:
        consts = ctx.enter_context(tc.tile_pool(name="consts", bufs=1))
        xpool = ctx.enter_context(tc.tile_pool(name="xpool", bufs=2))
        sqpool = ctx.enter_context(tc.tile_pool(name="sqpool", bufs=4))
        eqpool = ctx.enter_context(tc.tile_pool(name="eqpool", bufs=1))
        ycpool = ctx.enter_context(tc.tile_pool(name="ycpool", bufs=2))
        rspool = ctx.enter_context(tc.tile_pool(name="rspool", bufs=1))
        statp = ctx.enter_context(tc.tile_pool(name="statp", bufs=2))
        smallp = ctx.enter_context(tc.tile_pool(name="smallp", bufs=2))
        outp = ctx.enter_context(tc.tile_pool(name="outp", bufs=2))
        tinyp = ctx.enter_context(tc.tile_pool(name="tinyp", bufs=4))
        psA = ctx.enter_context(tc.tile_pool(name="psA", bufs=6, space="PSUM"))
        psC = ctx.enter_context(tc.tile_pool(name="psC", bufs=1, space="PSUM"))

        # constants into SBUF once
        wct_t = []
        for kt in range(2):
            t = consts.tile([128, 3 * INNER], f32r, tag=f"wct{kt}")
            nc.sync.dma_start(t[:], wct[kt * 128:(kt + 1) * 128, :])
            wct_t.append(t)
        woct_t = consts.tile([128, C], f32r, tag="woct")
        nc.sync.dma_start(woct_t[:], woct[:, :])
        boc_t = []
        for j in range(2):
            t = consts.tile([128, 1], f32, tag=f"boc{j}")
            nc.sync.dma_start(t[:], boc[j * 128:(j + 1) * 128, :])
            boc_t.append(t)
        ones_t = consts.tile([128, 128], f32r, tag="ones")
        nc.sync.dma_start(ones_t[:], onesc[:, :])
        hind_t = consts.tile([128, 128], f32r, tag="hind")
        nc.sync.dma_start(hind_t[:], hind[:, :])
        bmask_t = consts.tile([128, 128], f32, tag="bmask")
        nc.sync.dma_start(bmask_t[:], bmask[:, :])
        id_t = consts.tile([128, 128], f32r, tag="ident")
        nc.sync.dma_start(id_t[:], ident[:, :])
        eps_t = consts.tile([128, 1], f32, tag="eps")
        nc.vector.memset(eps_t[:], EPS)

        # PE "warm-up" touch of every constant: each matmul waits on exactly
        # one DMA lane, so no later PE instruction needs >1 sync wait.
        warm_ps = psA.tile([128, 128], f32, tag="pa")
        for t in (wct_t[0], wct_t[1], woct_t, ones_t, hind_t, bmask_t, id_t):
            nc.tensor.matmul(warm_ps[:, 0:1], t[:, 0:128].bitcast(f32),
                             t[:, 0:1].bitcast(f32), start=True, stop=True)
        for t in boc_t:
            nc.tensor.matmul(warm_ps[0:1, 0:1], t[:, 0:1], t[:, 0:1],
                             start=True, stop=True)

        NCH = 8          # 512-wide chunks
        CW = N // NCH    # 512

        for b in range(BPC):
            # ---- load x (2 c-tiles) ----
            xa = xpool.tile([128, N], f32r, tag="x")
            xb = xpool.tile([128, N], f32r, tag="x")
            nc.sync.dma_start(xa[:], xin[b, 0:128, :])
            nc.sync.dma_start(xb[:], xin[b, 128:256, :])

            # ---- LN1 stats -> rsig [128, N] broadcast ----
            rsig = rspool.tile([128, N], f32, tag="rsig")
            for ch in range(NCH):
                sl = bass.ts(ch, CW)
                xsq_a = sqpool.tile([128, CW], f32r, tag="sq")
                xsq_b = sqpool.tile([128, CW], f32r, tag="sq")
                nc.scalar.activation(xsq_a[:], xa[:, sl], AF.Square)
                nc.scalar.activation(xsq_b[:], xb[:, sl], AF.Square)
                mu_ps = psA.tile([128, CW], f32, tag="pa")
                nc.tensor.matmul(mu_ps[:], ones_t[:], xa[:, sl], start=True, stop=False)
                nc.tensor.matmul(mu_ps[:], ones_t[:], xb[:, sl], start=False, stop=True)
                msq_ps = psA.tile([128, CW], f32, tag="pa")
                nc.tensor.matmul(msq_ps[:], ones_t[:], xsq_a[:], start=True, stop=False)
                nc.tensor.matmul(msq_ps[:], ones_t[:], xsq_b[:], start=False, stop=True)
                musq = statp.tile([128, CW], f32, tag="st1")
                nc.scalar.activation(musq[:], mu_ps[:], AF.Square)
                var = statp.tile([128, CW], f32, tag="st2")
                nc.vector.tensor_tensor(var[:], msq_ps[:], musq[:], op=ALU.subtract)
                # 1/sqrt(var+eps) = exp(-0.5*ln(var+eps)): stays in the
                # natural_log_exp ACT table set (no table switch, no slow
                # DVE iterative reciprocal)
                lnv = statp.tile([128, CW], f32, tag="st3")
                nc.scalar.activation(lnv[:], var[:], AF.Ln, bias=eps_t[:])
                nc.scalar.activation(rsig[:, sl], lnv[:], AF.Exp, scale=-0.5)

            # ---- qkv + exp + transposes + context accumulation ----
            expq = eqpool.tile([128, N], f32r, tag="eq")
            ksum_parts = tinyp.tile([128, NCH], f32, tag="ksp")
            ctx_ps = psC.tile([128, 128], f32, tag="ctx")
            for ch in range(NCH):
                sl = bass.ts(ch, CW)
                xs_a = smallp.tile([128, CW], f32r, tag="xs")
                xs_b = smallp.tile([128, CW], f32r, tag="xs")
                nc.vector.tensor_mul(xs_a[:], xa[:, sl], rsig[:, sl])
                nc.vector.tensor_mul(xs_b[:], xb[:, sl], rsig[:, sl])

                q_ps = psA.tile([128, CW], f32, tag="pa")
                nc.tensor.matmul(q_ps[:], wct_t[0][:, 0:128], xs_a[:], start=True, stop=False)
                nc.tensor.matmul(q_ps[:], wct_t[1][:, 0:128], xs_b[:], start=False, stop=True)
                k_ps = psA.tile([128, CW], f32, tag="pa")
                nc.tensor.matmul(k_ps[:], wct_t[0][:, 128:256], xs_a[:], start=True, stop=False)
                nc.tensor.matmul(k_ps[:], wct_t[1][:, 128:256], xs_b[:], start=False, stop=True)
                v_ps = psA.tile([128, CW], f32, tag="pa")
                nc.tensor.matmul(v_ps[:], wct_t[0][:, 256:384], xs_a[:], start=True, stop=False)
                nc.tensor.matmul(v_ps[:], wct_t[1][:, 256:384], xs_b[:], start=False, stop=True)

                nc.scalar.activation(expq[:, sl], q_ps[:], AF.Exp)
                expk = smallp.tile([128, CW], f32r, tag="ek")
                nc.scalar.activation(expk[:], k_ps[:], AF.Exp,
                                     accum_out=ksum_parts[:, ch:ch + 1])
                v_sb = smallp.tile([128, CW], f32r, tag="vv")
                nc.scalar.copy(v_sb[:], v_ps[:])

                kT_ps = psA.tile([128, CW], f32r, tag="pa")
                vT_ps = psA.tile([128, CW], f32r, tag="pa")
                for j in range(4):
                    jl = bass.ts(j, 128)
                    nc.tensor.transpose(kT_ps[:, jl], expk[:, jl], id_t[:])
                    nc.tensor.transpose(vT_ps[:, jl], v_sb[:, jl], id_t[:])
                kT = smallp.tile([128, CW], f32r, tag="kt")
                nc.vector.tensor_copy(kT[:], kT_ps[:])
                vT = smallp.tile([128, CW], f32r, tag="vt")
                nc.vector.tensor_copy(vT[:], vT_ps[:])
                for j in range(4):
                    jl = bass.ts(j, 128)
                    nc.tensor.matmul(ctx_ps[:], kT[:, jl], vT[:, jl],
                                     start=(ch == 0 and j == 0),
                                     stop=(ch == NCH - 1 and j == 3))

            # ---- finish context: apply 1/ksum rows and scaled head mask ----
            ksum = tinyp.tile([128, 1], f32, tag="ks1")
            nc.vector.tensor_reduce(ksum[:], ksum_parts[:], axis=AX.X, op=ALU.add)
            rk = tinyp.tile([128, 1], f32, tag="rk")
            nc.vector.reciprocal(rk[:], ksum[:])
            ctx_a = tinyp.tile([128, 128], f32, tag="cxa")
            nc.vector.tensor_scalar_mul(ctx_a[:], ctx_ps[:], rk[:])
            ctx_m = tinyp.tile([128, 128], f32r, tag="cxm")
            nc.vector.tensor_mul(ctx_m[:], ctx_a[:], bmask_t[:])

            # ---- q normalization + out einsum + Wout ----
            yc_a = ycpool.tile([128, N], f32, tag="yc")
            yc_b = ycpool.tile([128, N], f32, tag="yc")
            for ch in range(NCH):
                sl = bass.ts(ch, CW)
                S_ps = psA.tile([128, CW], f32, tag="pa")
                nc.tensor.matmul(S_ps[:], hind_t[:], expq[:, sl], start=True, stop=True)
                rS = smallp.tile([128, CW], f32, tag="rs")
                nc.vector.reciprocal_approx_fast(rS[:], S_ps[:])
                o_ps = psA.tile([128, CW], f32, tag="pa")
                nc.tensor.matmul(o_ps[:], ctx_m[:], expq[:, sl], start=True, stop=True)
                attn = smallp.tile([128, CW], f32r, tag="at")
                nc.vector.tensor_mul(attn[:], o_ps[:], rS[:])
                y_ps0 = psA.tile([128, CW], f32, tag="pa")
                nc.tensor.matmul(y_ps0[:], woct_t[:, 0:128], attn[:], start=True, stop=True)
                y_ps1 = psA.tile([128, CW], f32, tag="pa")
                nc.tensor.matmul(y_ps1[:], woct_t[:, 128:256], attn[:], start=True, stop=True)
                nc.scalar.activation(yc_a[:, sl], y_ps0[:], AF.Identity, bias=boc_t[0][:])
                nc.scalar.activation(yc_b[:, sl], y_ps1[:], AF.Identity, bias=boc_t[1][:])

            # ---- LN2 (centered by construction) + residual ----
            for ch in range(NCH):
                sl = bass.ts(ch, CW)
                ysq_a = sqpool.tile([128, CW], f32r, tag="sq")
                ysq_b = sqpool.tile([128, CW], f32r, tag="sq")
                nc.scalar.activation(ysq_a[:], yc_a[:, sl], AF.Square)
                nc.scalar.activation(ysq_b[:], yc_b[:, sl], AF.Square)
                m2_ps = psA.tile([128, CW], f32, tag="pa")
                nc.tensor.matmul(m2_ps[:], ones_t[:], ysq_a[:], start=True, stop=False)
                nc.tensor.matmul(m2_ps[:], ones_t[:], ysq_b[:], start=False, stop=True)
                ln2 = statp.tile([128, CW], f32, tag="st1")
                nc.scalar.activation(ln2[:], m2_ps[:], AF.Ln, bias=eps_t[:])
                rsig2 = statp.tile([128, CW], f32, tag="st2")
                nc.scalar.activation(rsig2[:], ln2[:], AF.Exp, scale=-0.5)
                t_a = statp.tile([128, CW], f32, tag="st3")
                t_b = statp.tile([128, CW], f32, tag="st4")
                nc.vector.tensor_mul(t_a[:], yc_a[:, sl], rsig2[:])
                nc.vector.tensor_mul(t_b[:], yc_b[:, sl], rsig2[:])
                o_a = outp.tile([128, CW], f32, tag="oa")
                o_b = outp.tile([128, CW], f32, tag="ob")
                nc.vector.tensor_add(o_a[:], t_a[:], xa[:, sl])
                nc.vector.tensor_add(o_b[:], t_b[:], xb[:, sl])
                nc.sync.dma_start(out[b, 0:128, sl], o_a[:])
                nc.sync.dma_start(out[b, 128:256, sl], o_b[:])

    nc.compile()
    return nc


_CACHED = {}


def _get_nc():
    if "nc" not in _CACHED:
        _CACHED["nc"] = _build_bass()
    return _CACHED["nc"]


def kernel(x, Wqkv, Wout, bout):
    from concourse.bass_utils import run_bass_kernel_spmd

    x = np.ascontiguousarray(x, dtype=np.float32)
    Wqkv = np.asarray(Wqkv, dtype=np.float32)
    Wout = np.asarray(Wout, dtype=np.float32)
    bout = np.asarray(bout, dtype=np.float32)

    # host-side weight folding
    Wc = Wqkv - Wqkv.mean(axis=1, keepdims=True)          # centers LN1 input
    wct = np.ascontiguousarray(Wc.T)                      # [256, 384]
    Woc = Wout - Wout.mean(axis=0, keepdims=True)         # centers LN2 input
    woct = np.ascontiguousarray(Woc.T)                    # [128, 256]
    boc = (bout - bout.mean()).reshape(C, 1).astype(np.float32)

    onesc = np.full((128, 128), 1.0 / C, dtype=np.float32)
    r = np.arange(128)
    hind = (r[:, None] // DH == r[None, :] // DH).astype(np.float32)
    bmask = hind * np.float32(1.0 / (N * math.sqrt(DH)))
    ident = np.eye(128, dtype=np.float32)

    xr = x.reshape(B, C, N)
    nc = _get_nc()
    in_maps = []
    for core in range(NCORES):
        in_maps.append({
            "xin": np.ascontiguousarray(xr[core * BPC:(core + 1) * BPC]),
            "wct": wct, "woct": woct, "boc": boc,
            "onesc": onesc, "hind": hind, "bmask": bmask, "ident": ident,
        })
    res = run_bass_kernel_spmd(nc, in_maps, core_ids=list(range(NCORES)))
    outs = [res.results[c]["out"] for c in range(NCORES)]
    full = np.concatenate(outs, axis=0).reshape(B, C, 64, 64)
    return full


if __name__ == "__main__":
    rng = np.random.default_rng(0)
    x = rng.standard_normal((B, C, 64, 64), dtype=np.float32)
    Wqkv = rng.standard_normal((3 * INNER, C), dtype=np.float32)
    Wout = rng.standard_normal((C, INNER), dtype=np.float32)
    bout = rng.standard_normal((C,), dtype=np.float32)
    y = kernel(x=x, Wqkv=Wqkv, Wout=Wout, bout=bout)
    print(y.shape, y.dtype)



# revision 13
# speedup vs baseline: 1.9848x; 1.3976x over previous
"""Trainium2 Bass kernel for nn_AttnBlock (linear-attention block).

Full-input contract: kernel(**inputs) takes the complete arrays and returns the
complete output. Internally shards batch B=16 across 8 NeuronCores (2 each).

Math (per batch b, x_b [C=256, N=4096]):
  n1 = LN_C(x);  qkv = Wqkv @ n1;  q,k,v heads of 32
  q = softmax_d(q)/sqrt(32); k = softmax_N(k); v = v/N
  ctx_h = k_h @ v_h^T; out_h = ctx_h^T @ q_h
  y = Wout @ out + bout; out = LN_C(y) + x

Implementation notes:
  - All matmuls run in bf16 (1 cycle/row on the PE, fast weight loads);
    accumulation is always fp32 in PSUM. rel tolerance is 2e-2, bf16
    rounding keeps us well inside it.
  - LN mean-subtraction folded into host-centered weights.
  - k^T and v^T are computed directly transposed (xs as the stationary
    operand, W_kv columns moving), so no PE transposes are needed for the
    context matmul; the k-softmax denominator comes for free from a ones
    column appended to v^T (extra accumulator column in the ctx matmul).
  - 1/sqrt(var+eps) = exp(-0.5*ln(var+eps)): Ln and Exp live in the same
    ACT table set (natural_log_exp_and_others), so the scalar engine never
    pays the ~2.7us ACT_TABLE_LOAD switch that Sqrt would force; it also
    avoids the ~6 cycle/elem DVE iterative reciprocal.
  - x is loaded (and the output stored) with casting gpsimd DMAs, so all
    elementwise traffic on DVE runs in bf16 2x mode.
  - 1/N and 1/sqrt(32) folded into the per-head block mask applied to the
    context matrix; 1/ksum applied to the tiny [128,128] context matrix.
"""

import math
import numpy as np

HEADS = 4
DH = 32
C = 256
N = 4096
B = 16
NCORES = 8
BPC = B // NCORES  # batches per core
EPS = 1e-5
INNER = HEADS * DH  # 128
NB = N // 128       # 32 n-blocks per batch


def _build_bass():
    import concourse.bass as bass
    import concourse.bacc as bacc
    import concourse.tile as tile
    import concourse.mybir as mybir
    from contextlib import ExitStack

    f32 = mybir.dt.float32
    bf16 = mybir.dt.bfloat16
    AF = mybir.ActivationFunctionType
    ALU = mybir.AluOpType

    nc = bacc.Bacc("TRN2", target_bir_lowering=False, debug=False,
                   num_devices=NCORES)

    # DRAM I/O
    xin = nc.dram_tensor("xin", [BPC, C, N], f32, kind="ExternalInput")
    wq = nc.dram_tensor("wq", [C, 128], bf16, kind="ExternalInput")
    wkv = nc.dram_tensor("wkv", [C, 256], bf16, kind="ExternalInput")
    woct = nc.dram_tensor("woct", [INNER, C], bf16, kind="ExternalInput")
    boc = nc.dram_tensor("boc", [C, 1], f32, kind="ExternalInput")
    onesc = nc.dram_tensor("onesc", [128, 128], bf16, kind="ExternalInput")
    hind = nc.dram_tensor("hind", [128, 128], bf16, kind="ExternalInput")
    bmask = nc.dram_tensor("bmask", [128, 128], f32, kind="ExternalInput")
    out = nc.dram_tensor("out", [BPC, C, N], f32, kind="ExternalOutput")

    with tile.TileContext(nc) as tc, ExitStack() as ctx:
        consts = ctx.enter_context(tc.tile_pool(name="consts", bufs=1))
        xpool = ctx.enter_context(tc.tile_pool(name="xpool", bufs=3))
        sqpool = ctx.enter_context(tc.tile_pool(name="sqpool", bufs=4))
        f32pool = ctx.enter_context(tc.tile_pool(name="f32pool", bufs=1))
        rspool = ctx.enter_context(tc.tile_pool(name="rspool", bufs=1))
        xspool = ctx.enter_context(tc.tile_pool(name="xspool", bufs=2))
        eqpool = ctx.enter_context(tc.tile_pool(name="eqpool", bufs=1))
        kvpool = ctx.enter_context(tc.tile_pool(name="kvpool", bufs=1))
        atpool = ctx.enter_context(tc.tile_pool(name="atpool", bufs=1))
        ycpool = ctx.enter_context(tc.tile_pool(name="ycpool", bufs=2))
        obpool = ctx.enter_context(tc.tile_pool(name="obpool", bufs=2))
        statp = ctx.enter_context(tc.tile_pool(name="statp", bufs=3))
        tinyp = ctx.enter_context(tc.tile_pool(name="tinyp", bufs=4))
        psA = ctx.enter_context(tc.tile_pool(name="psA", bufs=5, space="PSUM"))
        psKV = ctx.enter_context(tc.tile_pool(name="psKV", bufs=2, space="PSUM"))
        psC = ctx.enter_context(tc.tile_pool(name="psC", bufs=1, space="PSUM"))

        # constants into SBUF once
        wq_t = []
        wkv_t = []
        for kt in range(2):
            t = consts.tile([128, 128], bf16, tag=f"wq{kt}")
            nc.sync.dma_start(t[:], wq[kt * 128:(kt + 1) * 128, :])
            wq_t.append(t)
            t = consts.tile([128, 256], bf16, tag=f"wkv{kt}")
            nc.sync.dma_start(t[:], wkv[kt * 128:(kt + 1) * 128, :])
            wkv_t.append(t)
        woct_t = consts.tile([128, C], bf16, tag="woct")
        nc.sync.dma_start(woct_t[:], woct[:, :])
        boc_t = []
        for j in range(2):
            t = consts.tile([128, 1], f32, tag=f"boc{j}")
            nc.sync.dma_start(t[:], boc[j * 128:(j + 1) * 128, :])
            boc_t.append(t)
        ones_t = consts.tile([128, 128], bf16, tag="ones")
        nc.sync.dma_start(ones_t[:], onesc[:, :])
        hind_t = consts.tile([128, 128], bf16, tag="hind")
        nc.sync.dma_start(hind_t[:], hind[:, :])
        bmask_t = consts.tile([128, 128], f32, tag="bmask")
        nc.sync.dma_start(bmask_t[:], bmask[:, :])
        eps_t = consts.tile([128, 1], f32, tag="eps")
        nc.vector.memset(eps_t[:], EPS)

        # PE warm-up touch of every matmul constant (one DMA wait each)
        warm_ps = psA.tile([128, 128], f32, tag="pa")
        for t in (wq_t[0], wq_t[1], wkv_t[0], wkv_t[1], woct_t, ones_t,
                  hind_t):
            nc.tensor.matmul(warm_ps[:, 0:2], t[:, 0:128], t[:, 0:2],
                             start=True, stop=True)

        NCH = 8          # 512-wide chunks
        CW = N // NCH    # 512

        for b in range(BPC):
            # ---- load x (2 c-tiles), casting fp32 -> bf16 in the DMA ----
            xa = xpool.tile([128, N], bf16, tag="x")
            xb = xpool.tile([128, N], bf16, tag="x")
            nc.gpsimd.dma_start(xa[:], xin[b, 0:128, :])
            nc.gpsimd.dma_start(xb[:], xin[b, 128:256, :])

            # ---- LN1 stats ----
            var_sb = f32pool.tile([128, N], f32, tag="var")
            for ch in range(NCH):
                sl = bass.ts(ch, CW)
                xsq_a = sqpool.tile([128, CW], bf16, tag="sq")
                xsq_b = sqpool.tile([128, CW], bf16, tag="sq")
                nc.vector.tensor_mul(xsq_a[:], xa[:, sl], xa[:, sl])
                nc.vector.tensor_mul(xsq_b[:], xb[:, sl], xb[:, sl])
                mu_ps = psA.tile([128, CW], f32, tag="pa")
                nc.tensor.matmul(mu_ps[:], ones_t[:], xa[:, sl], start=True, stop=False)
                nc.tensor.matmul(mu_ps[:], ones_t[:], xb[:, sl], start=False, stop=True)
                msq_ps = psA.tile([128, CW], f32, tag="pa")
                nc.tensor.matmul(msq_ps[:], ones_t[:], xsq_a[:], start=True, stop=False)
                nc.tensor.matmul(msq_ps[:], ones_t[:], xsq_b[:], start=False, stop=True)
                musq = statp.tile([128, CW], f32, tag="st1")
                nc.scalar.activation(musq[:], mu_ps[:], AF.Square)
                nc.vector.tensor_tensor(var_sb[:, sl], msq_ps[:], musq[:],
                                        op=ALU.subtract)
            # rsig = 1/sqrt(var+eps) = exp(-0.5*ln(var+eps)), full width
            nc.scalar.activation(var_sb[:], var_sb[:], AF.Ln, bias=eps_t[:])
            rsig = rspool.tile([128, N], bf16, tag="rsig")
            nc.scalar.activation(rsig[:], var_sb[:], AF.Exp, scale=-0.5)

            # ---- normalized x ----
            xs_a = xspool.tile([128, N], bf16, tag="xs")
            xs_b = xspool.tile([128, N], bf16, tag="xs")
            nc.vector.tensor_mul(xs_a[:], xa[:], rsig[:])
            nc.vector.tensor_mul(xs_b[:], xb[:], rsig[:])

            # ---- kT/vT computed directly transposed; ctx accumulation ----
            # kv_ps[n, 0:128] = k^T block, kv_ps[n, 128:256] = v^T block
            expkT = kvpool.tile([128, NB, 128], bf16, tag="ekt")
            vTs = kvpool.tile([128, NB, 129], bf16, tag="vts")
            nc.vector.memset(vTs[:, :, 128:129], 1.0)
            for bp in range(NB // 2):
                kv_ps = psKV.tile([128, 2, 256], f32, tag="kv")
                for i in range(2):
                    blk = 2 * bp + i
                    bsl = bass.ts(blk, 128)
                    nc.tensor.matmul(kv_ps[:, i, :], xs_a[:, bsl], wkv_t[0][:],
                                     start=True, stop=False)
                    nc.tensor.matmul(kv_ps[:, i, :], xs_b[:, bsl], wkv_t[1][:],
                                     start=False, stop=True)
                nc.scalar.activation(expkT[:, 2 * bp:2 * bp + 2, :],
                                     kv_ps[:, :, 0:128], AF.Exp)
                nc.vector.tensor_copy(vTs[:, 2 * bp:2 * bp + 2, 0:128],
                                      kv_ps[:, :, 128:256])
            ctx_ps = psC.tile([128, 129], f32, tag="ctx")
            for blk in range(NB):
                nc.tensor.matmul(ctx_ps[:], expkT[:, blk, :], vTs[:, blk, :],
                                 start=(blk == 0), stop=(blk == NB - 1))

            # ---- q path ----
            expq = eqpool.tile([128, N], bf16, tag="eq")
            for ch in range(NCH):
                sl = bass.ts(ch, CW)
                q_ps = psA.tile([128, CW], f32, tag="pa")
                nc.tensor.matmul(q_ps[:], wq_t[0][:], xs_a[:, sl], start=True, stop=False)
                nc.tensor.matmul(q_ps[:], wq_t[1][:], xs_b[:, sl], start=False, stop=True)
                nc.scalar.activation(expq[:, sl], q_ps[:], AF.Exp)

            # ---- finish context: 1/ksum rows and scaled head mask ----
            rk = tinyp.tile([128, 1], f32, tag="rk")
            nc.vector.reciprocal(rk[:], ctx_ps[:, 128:129])
            ctx_a = tinyp.tile([128, 128], f32, tag="cxa")
            nc.vector.tensor_scalar_mul(ctx_a[:], ctx_ps[:, 0:128], rk[:])
            ctx_m = tinyp.tile([128, 128], bf16, tag="cxm")
            nc.vector.tensor_mul(ctx_m[:], ctx_a[:], bmask_t[:])

            # ---- q normalization + out einsum + Wout ----
            attn = atpool.tile([128, N], bf16, tag="at")
            yc_a = ycpool.tile([128, N], bf16, tag="yc")
            yc_b = ycpool.tile([128, N], bf16, tag="yc")
            for ch in range(NCH):
                sl = bass.ts(ch, CW)
                S_ps = psA.tile([128, CW], f32, tag="pa")
                nc.tensor.matmul(S_ps[:], hind_t[:], expq[:, sl], start=True, stop=True)
                rS = statp.tile([128, CW], f32, tag="st2")
                nc.vector.reciprocal_approx_fast(rS[:], S_ps[:])
                o_ps = psA.tile([128, CW], f32, tag="pa")
                nc.tensor.matmul(o_ps[:], ctx_m[:], expq[:, sl], start=True, stop=True)
                nc.vector.tensor_mul(attn[:, sl], o_ps[:], rS[:])
                y_ps0 = psA.tile([128, CW], f32, tag="pa")
                nc.tensor.matmul(y_ps0[:], woct_t[:, 0:128], attn[:, sl], start=True, stop=True)
                y_ps1 = psA.tile([128, CW], f32, tag="pa")
                nc.tensor.matmul(y_ps1[:], woct_t[:, 128:256], attn[:, sl], start=True, stop=True)
                nc.scalar.activation(yc_a[:, sl], y_ps0[:], AF.Identity, bias=boc_t[0][:])
                nc.scalar.activation(yc_b[:, sl], y_ps1[:], AF.Identity, bias=boc_t[1][:])

            # ---- LN2 (centered by construction) + residual ----
            ysq_a = ycpool.tile([128, N], bf16, tag="ysq")
            ysq_b = ycpool.tile([128, N], bf16, tag="ysq")
            nc.vector.tensor_mul(ysq_a[:], yc_a[:], yc_a[:])
            nc.vector.tensor_mul(ysq_b[:], yc_b[:], yc_b[:])
            lnv2 = f32pool.tile([128, N], f32, tag="lnv2")
            for ch in range(NCH):
                sl = bass.ts(ch, CW)
                m2_ps = psA.tile([128, CW], f32, tag="pa")
                nc.tensor.matmul(m2_ps[:], ones_t[:], ysq_a[:, sl], start=True, stop=False)
                nc.tensor.matmul(m2_ps[:], ones_t[:], ysq_b[:, sl], start=False, stop=True)
                nc.scalar.activation(lnv2[:, sl], m2_ps[:], AF.Ln, bias=eps_t[:])
            rsig2 = rspool.tile([128, N], bf16, tag="rsig2")
            nc.scalar.activation(rsig2[:], lnv2[:], AF.Exp, scale=-0.5)
            t_a = ycpool.tile([128, N], bf16, tag="ysq")
            t_b = ycpool.tile([128, N], bf16, tag="ysq")
            nc.vector.tensor_mul(t_a[:], yc_a[:], rsig2[:])
            nc.vector.tensor_mul(t_b[:], yc_b[:], rsig2[:])
            ob_a = obpool.tile([128, N], bf16, tag="ob")
            ob_b = obpool.tile([128, N], bf16, tag="ob")
            nc.vector.tensor_add(ob_a[:], t_a[:], xa[:])
            nc.vector.tensor_add(ob_b[:], t_b[:], xb[:])
            # store with bf16 -> fp32 casting DMA
            nc.gpsimd.dma_start(out[b, 0:128, :], ob_a[:])
            nc.gpsimd.dma_start(out[b, 128:256, :], ob_b[:])

    nc.compile()
    return nc


_CACHED = {}


def _get_nc():
    if "nc" not in _CACHED:
        _CACHED["nc"] = _build_bass()
    return _CACHED["nc"]


def _make_in_maps(x, Wqkv, Wout, bout):
    import ml_dtypes

    bf = ml_dtypes.bfloat16
    x = np.ascontiguousarray(x, dtype=np.float32)
    Wqkv = np.asarray(Wqkv, dtype=np.float32)
    Wout = np.asarray(Wout, dtype=np.float32)
    bout = np.asarray(bout, dtype=np.float32)

    # host-side weight folding
    Wc = Wqkv - Wqkv.mean(axis=1, keepdims=True)          # centers LN1 input
    wct = np.ascontiguousarray(Wc.T)                      # [256, 384]
    wq = np.ascontiguousarray(wct[:, 0:128]).astype(bf)
    wkv = np.ascontiguousarray(wct[:, 128:384]).astype(bf)
    Woc = Wout - Wout.mean(axis=0, keepdims=True)         # centers LN2 input
    woct = np.ascontiguousarray(Woc.T).astype(bf)         # [128, 256]
    boc = (bout - bout.mean()).reshape(C, 1).astype(np.float32)

    onesc = np.full((128, 128), 1.0 / C, dtype=np.float32).astype(bf)
    r = np.arange(128)
    hindm = (r[:, None] // DH == r[None, :] // DH)
    hind = hindm.astype(bf)
    bmask = hindm.astype(np.float32) * np.float32(1.0 / (N * math.sqrt(DH)))

    xr = x.reshape(B, C, N)
    in_maps = []
    for core in range(NCORES):
        in_maps.append({
            "xin": np.ascontiguousarray(xr[core * BPC:(core + 1) * BPC]),
            "wq": wq, "wkv": wkv, "woct": woct, "boc": boc,
            "onesc": onesc, "hind": hind, "bmask": bmask,
        })
    return in_maps


def kernel(x, Wqkv, Wout, bout):
    from concourse.bass_utils import run_bass_kernel_spmd

    nc = _get_nc()
    in_maps = _make_in_maps(x, Wqkv, Wout, bout)
    res = run_bass_kernel_spmd(nc, in_maps, core_ids=list(range(NCORES)))
    outs = [res.results[c]["out"] for c in range(NCORES)]
    full = np.concatenate(outs, axis=0).reshape(B, C, 64, 64)
    return full


if __name__ == "__main__":
    rng = np.random.default_rng(0)
    x = rng.standard_normal((B, C, 64, 64), dtype=np.float32)
    Wqkv = rng.standard_normal((3 * INNER, C), dtype=np.float32)
    Wout = rng.standard_normal((C, INNER), dtype=np.float32)
    bout = rng.standard_normal((C,), dtype=np.float32)
    y = kernel(x=x, Wqkv=Wqkv, Wout=Wout, bout=bout)
    print(y.shape, y.dtype)
